# revision 26
# baseline (speedup 1.0000x reference)
"""Trainium2 Bass kernel for nn_AlignmentLoss (triplet + CE over phrase/input embeddings).

Sharding: batch dimension N=128 split 16 batches/core across 8 cores.  Each core
owns the positive pairs whose batch_idxs falls in its range (host buckets pairs,
padded to a fixed per-batch capacity cap=64; 2 batches share a 128-partition tile).

v9 design:
 - Host L2-normalizes phrase and input embeddings in f32 (exactly the
   reference's F.normalize preprocessing), so the device never computes
   norms.
 - Device computes the two big tensor contractions in fp8 (e4m3): sim
   rows -> DVE Max8 top-8 mining straight from PSUM, and CE logits ->
   ACT Exp(scale=T) with accum_out over the M phrases.  Per-pair stats
   (top-8 sims, sum-exp) DMA back; the host applies the O(P) hinge/log
   finale and the valid-pair masking/means.
 - DMA schedule: transfers within a queue complete round-robin, so the
   small stationaries get scalar's queue to themselves (earliest CE
   start), xt tiles 0-1 ride sync's queue, 2-4 ride pool's, and 5-7 are
   issued on scalar's queue anchored behind the first three Exps --
   after the smalls have drained, staggered so each rides mostly solo.
 - Output split in two DMAs so the first tiles' results stream out
   under the last Max8s.
"""

import sys

for _p in ("/opt/trn_rl_repo", "/root/.axon_site/_ro/trn_rl_repo"):
    if _p not in sys.path:
        sys.path.append(_p)

import numpy as np

import concourse.bass as bass
import concourse.bacc as bacc
import concourse.mybir as mybir
from concourse.tile import TileContext
from concourse.bass_utils import run_bass_kernel_spmd

F32 = mybir.dt.float32
BF16 = mybir.dt.bfloat16
FP8 = mybir.dt.float8e4
AF = mybir.ActivationFunctionType
ALU = mybir.AluOpType
AX = mybir.AxisListType

N, K, M, D, P = 128, 1024, 512, 128, 4096
NCORES = 8
NB = N // NCORES  # batches per core = 16


def build_graph(cap: int, T: float) -> bass.Bass:
    """One-core SPMD graph; cap = padded pairs per batch; T = temperature."""
    C = NB * cap          # padded pairs per core
    NT = C // 128         # 128-pair tiles
    BPT = 128 // cap      # batches per tile
    assert NT * 128 == C and BPT * cap == 128

    nc = bacc.Bacc(None, target_bir_lowering=False, debug=False)

    xt = nc.declare_dram_parameter("xt", [D, NB * K], FP8, isOutput=False)
    ancT = nc.declare_dram_parameter("ancT", [D, C], FP8, isOutput=False)
    posT = nc.declare_dram_parameter("posT", [D, C], FP8, isOutput=False)
    phrT = nc.declare_dram_parameter("phrT", [D, M], FP8, isOutput=False)
    out = nc.declare_dram_parameter("out", [128, 9 * NT], F32, isOutput=True)

    KB = BPT * K          # xt columns per tile-chunk (2048)

    with TileContext(nc) as tc:
        with (
            tc.tile_pool(name="big", bufs=1) as big,
            tc.tile_pool(name="work", bufs=2) as work,
            tc.tile_pool(name="prow", bufs=3, space="PSUM") as prow,
            tc.tile_pool(name="pce", bufs=2, space="PSUM") as pce,
        ):
            # ---- persistent tiles ----
            xt_sb = big.tile([128, NB * K], FP8, tag="xt")
            ancT_sb = big.tile([128, C], FP8, tag="ancT")
            posT_sb = big.tile([128, C], FP8, tag="posT")
            phrT_sb = big.tile([128, M], FP8, tag="phrT")
            out_sb = big.tile([128, 9 * NT], F32, tag="out")

            def xt_chunk(eng, t, half=None):
                lo = t * KB if half != 1 else t * KB + KB // 2
                hi = (t + 1) * KB if half != 0 else t * KB + KB // 2
                eng.dma_start(out=xt_sb[:, lo:hi], in_=xt[:, lo:hi])

            U32 = mybir.dt.uint32

            def gate_to(src_sb, src_ofs, dst_sb, dst_ofs):
                # Serialize DMA waves: transfers sharing a queue complete
                # round-robin, so a 2nd-wave chunk must not be issued while
                # the 1st wave is in flight.  This 4-byte copy reads the
                # 1st-wave dest (RAW on its DMA) and writes into the 2nd-wave
                # chunk's dest slice (WAW -> the 2nd DMA waits for it).
                dst = dst_sb[0:1, dst_ofs:dst_ofs + 4]
                src = src_sb[0:1, src_ofs:src_ofs + 4]
                nc.vector.tensor_copy(dst.bitcast(U32), src.bitcast(U32))

            def gate(src_sb, src_ofs, dst_t, slot):
                gate_to(src_sb, src_ofs, xt_sb, dst_t * KB + 4 * slot)

            # wave 1 -- pool q: tile 0 in halves (earliest sim start) + tile
            # 2; sync q: tile 1; scalar q: the small stationaries alone.
            # Wave 2 chunks are gated on wave-1 transfers (transfers sharing
            # a queue complete round-robin, so un-gated tails would starve
            # the heads).  Tuned to measured queue rates (pool/scalar ~110
            # B/ns, sync ~64 B/ns).
            PH = C // 2  # posT head: covers the first NT/2 CE tiles
            xt_chunk(nc.gpsimd, 0, half=0)
            xt_chunk(nc.gpsimd, 0, half=1)
            xt_chunk(nc.gpsimd, 2)
            xt_chunk(nc.sync, 1)
            nc.scalar.dma_start(out=posT_sb[:, 0:PH], in_=posT[:, 0:PH])
            nc.scalar.dma_start(out=phrT_sb, in_=phrT[:, :])
            nc.scalar.dma_start(out=ancT_sb, in_=ancT[:, :])
            # pool: {xt0,xt2} -> {xt4} -> {xt7}
            gate(xt_sb, 0, 4, 0)
            gate(xt_sb, KB // 2, 4, 1)
            gate(xt_sb, 2 * KB, 4, 2)
            xt_chunk(nc.gpsimd, 4)
            gate(xt_sb, 4 * KB, 7, 0)
            xt_chunk(nc.gpsimd, 7)
            # sync: {xt1} -> {xt6}
            gate(xt_sb, KB, 6, 0)
            xt_chunk(nc.sync, 6)
            # scalar: {posT_h, phrT, ancT} -> {xt3, posT_tail} -> {xt5}
            gate(ancT_sb, 0, 3, 0)
            gate(posT_sb, 0, 3, 1)
            gate(phrT_sb, 0, 3, 2)
            xt_chunk(nc.scalar, 3)
            gate_to(ancT_sb, 4, posT_sb, PH)
            nc.scalar.dma_start(out=posT_sb[:, PH:C], in_=posT[:, PH:C])
            gate(xt_sb, 3 * KB, 5, 0)
            xt_chunk(nc.scalar, 5)

            def ce_mm(t):
                lg = pce.tile([128, 512], F32, tag="lg")
                nc.tensor.matmul(lg, posT_sb[:, t * 128:(t + 1) * 128],
                                 phrT_sb, start=True, stop=True)
                je = work.tile([128, 512], BF16, tag="je")
                nc.scalar.activation(je, lg, AF.Exp, scale=float(T),
                                     accum_out=out_sb[:, 8 * NT + t:8 * NT + t + 1])

            def sim_mm(t):
                rp = prow.tile([128, 1024], F32, tag="rp")
                for h in range(BPT):
                    b = BPT * t + h
                    acols = ancT_sb[:, b * cap:(b + 1) * cap]
                    for g in range(K // 512):
                        nc.tensor.matmul(
                            rp[cap * h:cap * (h + 1), g * 512:(g + 1) * 512],
                            acols,
                            xt_sb[:, b * K + g * 512:b * K + (g + 1) * 512],
                            start=True, stop=True)
                nc.vector.max(out_sb[:, t * 8:(t + 1) * 8], rp)

            # PE order: a few CE matmuls first (tiny DMA deps; they warm the
            # p-state), then interleave sims as xt chunks land.
            ce_mm(0); ce_mm(1); ce_mm(2)
            nxt = 3
            for t in range(NT):
                sim_mm(t)
                if nxt < NT:
                    ce_mm(nxt)
                    nxt += 1

            # stream out the first six tiles under the last Max8s
            nc.sync.dma_start(out=out[:, 0:48], in_=out_sb[:, 0:48])
            nc.sync.dma_start(out=out[:, 48:9 * NT], in_=out_sb[:, 48:9 * NT])

    if not nc.is_finalized():
        nc.finalize()
    return nc


_CACHE = {}
_FP8 = mybir.dt.np(FP8)


def _l2n(x):
    return x / np.maximum(np.linalg.norm(x, axis=-1, keepdims=True), 1e-12)


def _prep_core(c, cap, pe, ie, bi, mi, ki, rn):
    """pe/ie are pre-normalized f32.  Returns (device map, host-side stats)."""
    C = NB * cap
    NT = C // 128
    lo = NB * c
    sel = np.where((bi >= lo) & (bi < lo + NB))[0]
    # pad with unit vectors (already normalized)
    ancb = np.zeros((C, D), np.float32); ancb[:, 0] = 1.0
    posb = np.zeros((C, D), np.float32); posb[:, 0] = 1.0
    rngb = np.zeros((C, 2, D), np.float32); rngb[:, :, 0] = 1.0
    valid = np.zeros(C, np.float32)
    for n in range(NB):
        pb = sel[bi[sel] == lo + n]
        assert len(pb) <= cap
        s = n * cap
        ancb[s:s + len(pb)] = pe[mi[pb]]
        posb[s:s + len(pb)] = ie[bi[pb], ki[pb]]
        rngb[s:s + len(pb), 0] = ie[bi[pb], rn[pb, 0]]
        rngb[s:s + len(pb), 1] = ie[bi[pb], rn[pb, 1]]
        valid[s:s + len(pb)] = 1.0
    xt_c = np.ascontiguousarray(
        ie[lo:lo + NB].reshape(NB * K, D).T).astype(_FP8)
    dev = dict(
        xt=xt_c,
        ancT=np.ascontiguousarray(ancb.T).astype(_FP8),
        posT=np.ascontiguousarray(posb.T).astype(_FP8),
        phrT=np.ascontiguousarray(pe.T).astype(_FP8),
    )
    # host-side per-pair stats in [128, NT] tile layout (tile t, partition p
    # <-> pair t*128+p), matching the device's Max8 output layout
    spos = np.einsum('cd,cd->c', ancb, posb).reshape(NT, 128).T
    srnd = np.einsum('cd,crd->cr', ancb, rngb).reshape(NT, 128, 2).transpose(1, 0, 2)
    vt = valid.reshape(NT, 128).T
    return dev, (spos, srnd, vt)


def make_in_maps(inputs, cap=None):
    pe = _l2n(np.asarray(inputs["phrase_embeddings"], np.float32))
    ie = _l2n(np.asarray(inputs["input_embeddings"], np.float32))
    bi = np.asarray(inputs["batch_idxs"])
    mi = np.asarray(inputs["phrase_emb_idxs"])
    ki = np.asarray(inputs["input_emb_idxs"])
    rn = np.asarray(inputs["rand_neg_idx"])
    T = float(np.asarray(inputs["temperature"]))
    if cap is None:
        maxc = int(np.bincount(bi, minlength=N).max())
        cap = max(64, ((maxc + 63) // 64) * 64)
    maps, stats = [], []
    for c in range(NCORES):
        m, st = _prep_core(c, cap, pe, ie, bi, mi, ki, rn)
        maps.append(m)
        stats.append(st)
    return maps, stats, cap, T


def kernel(**inputs):
    in_maps, stats, cap, T = make_in_maps(inputs)
    key = (cap, T)
    if key not in _CACHE:
        _CACHE[key] = build_graph(cap, T)
    nc = _CACHE[key]
    res = run_bass_kernel_spmd(nc, in_maps, core_ids=list(range(NCORES)))
    NT = NB * cap // 128
    trip_sum = 0.0
    ce_sum = 0.0
    for c, r in enumerate(res.results):
        of = np.asarray(r["out"], np.float32)            # [128, 9NT]
        t8 = of[:, :8 * NT].reshape(128, NT, 8)
        sumexp = of[:, 8 * NT:9 * NT]                    # [128, NT]
        spos, srnd, vt = stats[c]
        u = np.maximum(t8[:, :, :4] + 1.0 - spos[:, :, None], 0.0)
        s4 = u.sum(-1)
        w = np.maximum(u[:, :, 3], 1.0)
        r2 = np.maximum(srnd + 1.0 - spos[:, :, None], 0.0).sum(-1)
        trip_sum += float(((s4 - w + r2) * vt).sum())
        ce_sum += float(((np.log(sumexp) - T * spos) * vt).sum())
    trip = trip_sum / (P * 5)
    ce = ce_sum / P
    return np.float32(trip), np.float32(ce)


# revision 27
# speedup vs baseline: 1.0017x; 1.0017x over previous
"""Trainium2 Bass kernel for nn_AlignmentLoss (triplet + CE over phrase/input embeddings).

Sharding: batch dimension N=128 split 16 batches/core across 8 cores.  Each core
owns the positive pairs whose batch_idxs falls in its range (host buckets pairs,
padded to a fixed per-batch capacity cap=64; 2 batches share a 128-partition tile).

v9 design:
 - Host L2-normalizes phrase and input embeddings in f32 (exactly the
   reference's F.normalize preprocessing), so the device never computes
   norms.
 - Device computes the two big tensor contractions in fp8 (e4m3): sim
   rows -> DVE Max8 top-8 mining straight from PSUM, and CE logits ->
   ACT Exp(scale=T) with accum_out over the M phrases.  Per-pair stats
   (top-8 sims, sum-exp) DMA back; the host applies the O(P) hinge/log
   finale and the valid-pair masking/means.
 - DMA schedule: transfers within a queue complete round-robin, so the
   small stationaries get scalar's queue to themselves (earliest CE
   start), xt tiles 0-1 ride sync's queue, 2-4 ride pool's, and 5-7 are
   issued on scalar's queue anchored behind the first three Exps --
   after the smalls have drained, staggered so each rides mostly solo.
 - Output split in two DMAs so the first tiles' results stream out
   under the last Max8s.
"""

import sys

for _p in ("/opt/trn_rl_repo", "/root/.axon_site/_ro/trn_rl_repo"):
    if _p not in sys.path:
        sys.path.append(_p)

import numpy as np

import concourse.bass as bass
import concourse.bacc as bacc
import concourse.mybir as mybir
from concourse.tile import TileContext
from concourse.bass_utils import run_bass_kernel_spmd

F32 = mybir.dt.float32
BF16 = mybir.dt.bfloat16
FP8 = mybir.dt.float8e4
AF = mybir.ActivationFunctionType
ALU = mybir.AluOpType
AX = mybir.AxisListType

N, K, M, D, P = 128, 1024, 512, 128, 4096
NCORES = 8
NB = N // NCORES  # batches per core = 16


def build_graph(cap: int, T: float) -> bass.Bass:
    """One-core SPMD graph; cap = padded pairs per batch; T = temperature."""
    C = NB * cap          # padded pairs per core
    NT = C // 128         # 128-pair tiles
    BPT = 128 // cap      # batches per tile
    assert NT * 128 == C and BPT * cap == 128

    nc = bacc.Bacc(None, target_bir_lowering=False, debug=False)

    xt = nc.declare_dram_parameter("xt", [D, NB * K], FP8, isOutput=False)
    ancT = nc.declare_dram_parameter("ancT", [D, C], FP8, isOutput=False)
    posT = nc.declare_dram_parameter("posT", [D, C], FP8, isOutput=False)
    phrT = nc.declare_dram_parameter("phrT", [D, M], FP8, isOutput=False)
    out = nc.declare_dram_parameter("out", [128, 9 * NT], F32, isOutput=True)

    KB = BPT * K          # xt columns per tile-chunk (2048)

    with TileContext(nc) as tc:
        with (
            tc.tile_pool(name="big", bufs=1) as big,
            tc.tile_pool(name="work", bufs=2) as work,
            tc.tile_pool(name="prow", bufs=3, space="PSUM") as prow,
            tc.tile_pool(name="pce", bufs=2, space="PSUM") as pce,
        ):
            # ---- persistent tiles ----
            xt_sb = big.tile([128, NB * K], FP8, tag="xt")
            ancT_sb = big.tile([128, C], FP8, tag="ancT")
            posT_sb = big.tile([128, C], FP8, tag="posT")
            phrT_sb = big.tile([128, M], FP8, tag="phrT")
            out_sb = big.tile([128, 9 * NT], F32, tag="out")

            def xt_chunk(eng, t, half=None):
                lo = t * KB if half != 1 else t * KB + KB // 2
                hi = (t + 1) * KB if half != 0 else t * KB + KB // 2
                eng.dma_start(out=xt_sb[:, lo:hi], in_=xt[:, lo:hi])

            U32 = mybir.dt.uint32

            def gate_to(src_sb, src_ofs, dst_sb, dst_ofs):
                # Serialize DMA waves: transfers sharing a queue complete
                # round-robin, so a 2nd-wave chunk must not be issued while
                # the 1st wave is in flight.  This 4-byte copy reads the
                # 1st-wave dest (RAW on its DMA) and writes into the 2nd-wave
                # chunk's dest slice (WAW -> the 2nd DMA waits for it).
                dst = dst_sb[0:1, dst_ofs:dst_ofs + 4]
                src = src_sb[0:1, src_ofs:src_ofs + 4]
                nc.vector.tensor_copy(dst.bitcast(U32), src.bitcast(U32))

            def gate(src_sb, src_ofs, dst_t, slot):
                gate_to(src_sb, src_ofs, xt_sb, dst_t * KB + 4 * slot)

            # wave 1 -- pool q: tile 0 in halves (earliest sim start) + tile
            # 2; sync q: tile 1; scalar q: the small stationaries alone.
            # Wave 2 chunks are gated on wave-1 transfers (transfers sharing
            # a queue complete round-robin, so un-gated tails would starve
            # the heads).  Tuned to measured queue rates (pool/scalar ~110
            # B/ns, sync ~64 B/ns).
            PH = C // 2  # posT head: covers the first NT/2 CE tiles
            xt_chunk(nc.gpsimd, 0, half=0)
            xt_chunk(nc.gpsimd, 0, half=1)
            xt_chunk(nc.gpsimd, 2)
            xt_chunk(nc.sync, 1)
            nc.scalar.dma_start(out=posT_sb[:, 0:PH], in_=posT[:, 0:PH])
            nc.scalar.dma_start(out=phrT_sb, in_=phrT[:, :])
            nc.scalar.dma_start(out=ancT_sb, in_=ancT[:, :])
            # pool: {xt0,xt2} -> {xt6, xt7}
            gate(xt_sb, 0, 6, 0)
            gate(xt_sb, KB // 2, 6, 1)
            gate(xt_sb, 2 * KB, 7, 0)
            xt_chunk(nc.gpsimd, 6)
            xt_chunk(nc.gpsimd, 7)
            # sync: {xt1} -> {xt4}
            gate(xt_sb, KB, 4, 0)
            xt_chunk(nc.sync, 4)
            # scalar: {posT_h, phrT, ancT} -> {xt3, posT_tail} -> {xt5}
            gate(ancT_sb, 0, 3, 0)
            gate(posT_sb, 0, 3, 1)
            gate(phrT_sb, 0, 3, 2)
            xt_chunk(nc.scalar, 3)
            gate_to(ancT_sb, 4, posT_sb, PH)
            nc.scalar.dma_start(out=posT_sb[:, PH:C], in_=posT[:, PH:C])
            gate(xt_sb, 3 * KB, 5, 0)
            xt_chunk(nc.scalar, 5)

            def ce_mm(t):
                lg = pce.tile([128, 512], F32, tag="lg")
                nc.tensor.matmul(lg, posT_sb[:, t * 128:(t + 1) * 128],
                                 phrT_sb, start=True, stop=True)
                je = work.tile([128, 512], BF16, tag="je")
                nc.scalar.activation(je, lg, AF.Exp, scale=float(T),
                                     accum_out=out_sb[:, 8 * NT + t:8 * NT + t + 1])

            def sim_mm(t):
                rp = prow.tile([128, 1024], F32, tag="rp")
                for h in range(BPT):
                    b = BPT * t + h
                    acols = ancT_sb[:, b * cap:(b + 1) * cap]
                    for g in range(K // 512):
                        nc.tensor.matmul(
                            rp[cap * h:cap * (h + 1), g * 512:(g + 1) * 512],
                            acols,
                            xt_sb[:, b * K + g * 512:b * K + (g + 1) * 512],
                            start=True, stop=True)
                nc.vector.max(out_sb[:, t * 8:(t + 1) * 8], rp)

            # PE order: a few CE matmuls first (tiny DMA deps; they warm the
            # p-state), then interleave sims as xt chunks land.
            ce_mm(0); ce_mm(1); ce_mm(2)
            nxt = 3
            for t in range(NT):
                sim_mm(t)
                if nxt < NT:
                    ce_mm(nxt)
                    nxt += 1

            # stream out the first six tiles under the last Max8s
            nc.sync.dma_start(out=out[:, 0:48], in_=out_sb[:, 0:48])
            nc.sync.dma_start(out=out[:, 48:9 * NT], in_=out_sb[:, 48:9 * NT])

    if not nc.is_finalized():
        nc.finalize()
    return nc


_CACHE = {}
_FP8 = mybir.dt.np(FP8)


def _l2n(x):
    return x / np.maximum(np.linalg.norm(x, axis=-1, keepdims=True), 1e-12)


def _prep_core(c, cap, pe, ie, bi, mi, ki, rn):
    """pe/ie are pre-normalized f32.  Returns (device map, host-side stats)."""
    C = NB * cap
    NT = C // 128
    lo = NB * c
    sel = np.where((bi >= lo) & (bi < lo + NB))[0]
    # pad with unit vectors (already normalized)
    ancb = np.zeros((C, D), np.float32); ancb[:, 0] = 1.0
    posb = np.zeros((C, D), np.float32); posb[:, 0] = 1.0
    rngb = np.zeros((C, 2, D), np.float32); rngb[:, :, 0] = 1.0
    valid = np.zeros(C, np.float32)
    for n in range(NB):
        pb = sel[bi[sel] == lo + n]
        assert len(pb) <= cap
        s = n * cap
        ancb[s:s + len(pb)] = pe[mi[pb]]
        posb[s:s + len(pb)] = ie[bi[pb], ki[pb]]
        rngb[s:s + len(pb), 0] = ie[bi[pb], rn[pb, 0]]
        rngb[s:s + len(pb), 1] = ie[bi[pb], rn[pb, 1]]
        valid[s:s + len(pb)] = 1.0
    xt_c = np.ascontiguousarray(
        ie[lo:lo + NB].reshape(NB * K, D).T).astype(_FP8)
    dev = dict(
        xt=xt_c,
        ancT=np.ascontiguousarray(ancb.T).astype(_FP8),
        posT=np.ascontiguousarray(posb.T).astype(_FP8),
        phrT=np.ascontiguousarray(pe.T).astype(_FP8),
    )
    # host-side per-pair stats in [128, NT] tile layout (tile t, partition p
    # <-> pair t*128+p), matching the device's Max8 output layout
    spos = np.einsum('cd,cd->c', ancb, posb).reshape(NT, 128).T
    srnd = np.einsum('cd,crd->cr', ancb, rngb).reshape(NT, 128, 2).transpose(1, 0, 2)
    vt = valid.reshape(NT, 128).T
    return dev, (spos, srnd, vt)


def make_in_maps(inputs, cap=None):
    pe = _l2n(np.asarray(inputs["phrase_embeddings"], np.float32))
    ie = _l2n(np.asarray(inputs["input_embeddings"], np.float32))
    bi = np.asarray(inputs["batch_idxs"])
    mi = np.asarray(inputs["phrase_emb_idxs"])
    ki = np.asarray(inputs["input_emb_idxs"])
    rn = np.asarray(inputs["rand_neg_idx"])
    T = float(np.asarray(inputs["temperature"]))
    if cap is None:
        maxc = int(np.bincount(bi, minlength=N).max())
        cap = max(64, ((maxc + 63) // 64) * 64)
    maps, stats = [], []
    for c in range(NCORES):
        m, st = _prep_core(c, cap, pe, ie, bi, mi, ki, rn)
        maps.append(m)
        stats.append(st)
    return maps, stats, cap, T


def kernel(**inputs):
    in_maps, stats, cap, T = make_in_maps(inputs)
    key = (cap, T)
    if key not in _CACHE:
        _CACHE[key] = build_graph(cap, T)
    nc = _CACHE[key]
    res = run_bass_kernel_spmd(nc, in_maps, core_ids=list(range(NCORES)))
    NT = NB * cap // 128
    trip_sum = 0.0
    ce_sum = 0.0
    for c, r in enumerate(res.results):
        of = np.asarray(r["out"], np.float32)            # [128, 9NT]
        t8 = of[:, :8 * NT].reshape(128, NT, 8)
        sumexp = of[:, 8 * NT:9 * NT]                    # [128, NT]
        spos, srnd, vt = stats[c]
        u = np.maximum(t8[:, :, :4] + 1.0 - spos[:, :, None], 0.0)
        s4 = u.sum(-1)
        w = np.maximum(u[:, :, 3], 1.0)
        r2 = np.maximum(srnd + 1.0 - spos[:, :, None], 0.0).sum(-1)
        trip_sum += float(((s4 - w + r2) * vt).sum())
        ce_sum += float(((np.log(sumexp) - T * spos) * vt).sum())
    trip = trip_sum / (P * 5)
    ce = ce_sum / P
    return np.float32(trip), np.float32(ce)


# revision 28
# speedup vs baseline: 1.0805x; 1.0787x over previous
"""Trainium2 Bass kernel for nn_AlignmentLoss (triplet + CE over phrase/input embeddings).

Sharding: batch dimension N=128 split 16 batches/core across 8 cores.  Each core
owns the positive pairs whose batch_idxs falls in its range (host buckets pairs,
padded to a fixed per-batch capacity cap=64; 2 batches share a 128-partition tile).

v9 design:
 - Host L2-normalizes phrase and input embeddings in f32 (exactly the
   reference's F.normalize preprocessing), so the device never computes
   norms.
 - Device computes the two big tensor contractions in fp8 (e4m3): sim
   rows -> DVE Max8 top-8 mining straight from PSUM, and CE logits ->
   ACT Exp(scale=T) with accum_out over the M phrases.  Per-pair stats
   (top-8 sims, sum-exp) DMA back; the host applies the O(P) hinge/log
   finale and the valid-pair masking/means.
 - DMA schedule: transfers within a queue complete round-robin, so the
   small stationaries get scalar's queue to themselves (earliest CE
   start), xt tiles 0-1 ride sync's queue, 2-4 ride pool's, and 5-7 are
   issued on scalar's queue anchored behind the first three Exps --
   after the smalls have drained, staggered so each rides mostly solo.
 - Output split in two DMAs so the first tiles' results stream out
   under the last Max8s.
"""

import sys

for _p in ("/opt/trn_rl_repo", "/root/.axon_site/_ro/trn_rl_repo"):
    if _p not in sys.path:
        sys.path.append(_p)

import numpy as np

import concourse.bass as bass
import concourse.bacc as bacc
import concourse.mybir as mybir
from concourse.tile import TileContext
from concourse.bass_utils import run_bass_kernel_spmd

F32 = mybir.dt.float32
BF16 = mybir.dt.bfloat16
FP8 = mybir.dt.float8e4
AF = mybir.ActivationFunctionType
ALU = mybir.AluOpType
AX = mybir.AxisListType

N, K, M, D, P = 128, 1024, 512, 128, 4096
NCORES = 8
NB = N // NCORES  # batches per core = 16


def build_graph(cap: int, T: float) -> bass.Bass:
    """One-core SPMD graph; cap = padded pairs per batch; T = temperature."""
    C = NB * cap          # padded pairs per core
    NT = C // 128         # 128-pair tiles
    BPT = 128 // cap      # batches per tile
    assert NT * 128 == C and BPT * cap == 128

    nc = bacc.Bacc(None, target_bir_lowering=False, debug=False)

    xt = nc.declare_dram_parameter("xt", [D, NB * K], FP8, isOutput=False)
    ancT = nc.declare_dram_parameter("ancT", [D, C], FP8, isOutput=False)
    posT = nc.declare_dram_parameter("posT", [D, C], FP8, isOutput=False)
    phrT = nc.declare_dram_parameter("phrT", [D, M], FP8, isOutput=False)
    out = nc.declare_dram_parameter("out", [128, 9 * NT], F32, isOutput=True)

    KB = BPT * K          # xt columns per tile-chunk (2048)

    with TileContext(nc) as tc:
        with (
            tc.tile_pool(name="big", bufs=1) as big,
            tc.tile_pool(name="work", bufs=2) as work,
            tc.tile_pool(name="prow", bufs=3, space="PSUM") as prow,
            tc.tile_pool(name="pce", bufs=2, space="PSUM") as pce,
        ):
            # ---- persistent tiles ----
            xt_sb = big.tile([128, NB * K], FP8, tag="xt")
            ancT_sb = big.tile([128, C], FP8, tag="ancT")
            posT_sb = big.tile([128, C], FP8, tag="posT")
            phrT_sb = big.tile([128, M], FP8, tag="phrT")
            out_sb = big.tile([128, 9 * NT], F32, tag="out")

            def xt_chunk(eng, t, half=None):
                lo = t * KB if half != 1 else t * KB + KB // 2
                hi = (t + 1) * KB if half != 0 else t * KB + KB // 2
                eng.dma_start(out=xt_sb[:, lo:hi], in_=xt[:, lo:hi])

            U32 = mybir.dt.uint32

            def gate_to(src_sb, src_ofs, dst_sb, dst_ofs):
                # Serialize DMA waves: transfers sharing a queue complete
                # round-robin, so a 2nd-wave chunk must not be issued while
                # the 1st wave is in flight.  This 4-byte copy reads the
                # 1st-wave dest (RAW on its DMA) and writes into the 2nd-wave
                # chunk's dest slice (WAW -> the 2nd DMA waits for it).
                dst = dst_sb[0:1, dst_ofs:dst_ofs + 4]
                src = src_sb[0:1, src_ofs:src_ofs + 4]
                nc.vector.tensor_copy(dst.bitcast(U32), src.bitcast(U32))

            def gate(src_sb, src_ofs, dst_t, slot):
                gate_to(src_sb, src_ofs, xt_sb, dst_t * KB + 4 * slot)

            # wave 1 -- pool q: tile 0 in halves (earliest sim start) + tile
            # 2; sync q: tile 1; scalar q: the small stationaries alone.
            # Wave 2 chunks are gated on wave-1 transfers (transfers sharing
            # a queue complete round-robin, so un-gated tails would starve
            # the heads).  Tuned to measured queue rates (pool/scalar ~110
            # B/ns, sync ~64 B/ns).
            xt_chunk(nc.gpsimd, 0, half=0)
            xt_chunk(nc.gpsimd, 0, half=1)
            xt_chunk(nc.gpsimd, 2)
            xt_chunk(nc.sync, 1)
            nc.scalar.dma_start(out=posT_sb, in_=posT[:, :])
            nc.scalar.dma_start(out=phrT_sb, in_=phrT[:, :])
            nc.scalar.dma_start(out=ancT_sb, in_=ancT[:, :])
            # pool: {xt0,xt2} -> {xt6, xt7}
            gate(xt_sb, 0, 6, 0)
            gate(xt_sb, KB // 2, 6, 1)
            gate(xt_sb, 2 * KB, 7, 0)
            xt_chunk(nc.gpsimd, 6)
            xt_chunk(nc.gpsimd, 7)
            # sync: {xt1} -> {xt4}
            gate(xt_sb, KB, 4, 0)
            xt_chunk(nc.sync, 4)
            # scalar: {posT_h, phrT, ancT} -> {xt3, posT_tail} -> {xt5}
            gate(ancT_sb, 0, 3, 0)
            gate(posT_sb, 0, 3, 1)
            gate(phrT_sb, 0, 3, 2)
            xt_chunk(nc.scalar, 3)
            gate(xt_sb, 3 * KB, 5, 0)
            xt_chunk(nc.scalar, 5)

            def ce_mm(t):
                lg = pce.tile([128, 512], F32, tag="lg")
                nc.tensor.matmul(lg, posT_sb[:, t * 128:(t + 1) * 128],
                                 phrT_sb, start=True, stop=True)
                je = work.tile([128, 512], BF16, tag="je")
                nc.scalar.activation(je, lg, AF.Exp, scale=float(T),
                                     accum_out=out_sb[:, 8 * NT + t:8 * NT + t + 1])

            def sim_mm(t):
                rp = prow.tile([128, 1024], F32, tag="rp")
                for h in range(BPT):
                    b = BPT * t + h
                    acols = ancT_sb[:, b * cap:(b + 1) * cap]
                    for g in range(K // 512):
                        nc.tensor.matmul(
                            rp[cap * h:cap * (h + 1), g * 512:(g + 1) * 512],
                            acols,
                            xt_sb[:, b * K + g * 512:b * K + (g + 1) * 512],
                            start=True, stop=True)
                nc.vector.max(out_sb[:, t * 8:(t + 1) * 8], rp)

            # PE order: a few CE matmuls first (tiny DMA deps; they warm the
            # p-state), then interleave sims as xt chunks land.
            ce_mm(0); ce_mm(1); ce_mm(2)
            nxt = 3
            for t in range(NT):
                sim_mm(t)
                if nxt < NT:
                    ce_mm(nxt)
                    nxt += 1

            # stream out the first six tiles under the last Max8s
            nc.sync.dma_start(out=out[:, 0:48], in_=out_sb[:, 0:48])
            nc.sync.dma_start(out=out[:, 48:9 * NT], in_=out_sb[:, 48:9 * NT])

    if not nc.is_finalized():
        nc.finalize()
    return nc


_CACHE = {}
_FP8 = mybir.dt.np(FP8)


def _l2n(x):
    return x / np.maximum(np.linalg.norm(x, axis=-1, keepdims=True), 1e-12)


def _prep_core(c, cap, pe, ie, bi, mi, ki, rn):
    """pe/ie are pre-normalized f32.  Returns (device map, host-side stats)."""
    C = NB * cap
    NT = C // 128
    lo = NB * c
    sel = np.where((bi >= lo) & (bi < lo + NB))[0]
    # pad with unit vectors (already normalized)
    ancb = np.zeros((C, D), np.float32); ancb[:, 0] = 1.0
    posb = np.zeros((C, D), np.float32); posb[:, 0] = 1.0
    rngb = np.zeros((C, 2, D), np.float32); rngb[:, :, 0] = 1.0
    valid = np.zeros(C, np.float32)
    for n in range(NB):
        pb = sel[bi[sel] == lo + n]
        assert len(pb) <= cap
        s = n * cap
        ancb[s:s + len(pb)] = pe[mi[pb]]
        posb[s:s + len(pb)] = ie[bi[pb], ki[pb]]
        rngb[s:s + len(pb), 0] = ie[bi[pb], rn[pb, 0]]
        rngb[s:s + len(pb), 1] = ie[bi[pb], rn[pb, 1]]
        valid[s:s + len(pb)] = 1.0
    xt_c = np.ascontiguousarray(
        ie[lo:lo + NB].reshape(NB * K, D).T).astype(_FP8)
    dev = dict(
        xt=xt_c,
        ancT=np.ascontiguousarray(ancb.T).astype(_FP8),
        posT=np.ascontiguousarray(posb.T).astype(_FP8),
        phrT=np.ascontiguousarray(pe.T).astype(_FP8),
    )
    # host-side per-pair stats in [128, NT] tile layout (tile t, partition p
    # <-> pair t*128+p), matching the device's Max8 output layout
    spos = np.einsum('cd,cd->c', ancb, posb).reshape(NT, 128).T
    srnd = np.einsum('cd,crd->cr', ancb, rngb).reshape(NT, 128, 2).transpose(1, 0, 2)
    vt = valid.reshape(NT, 128).T
    return dev, (spos, srnd, vt)


def make_in_maps(inputs, cap=None):
    pe = _l2n(np.asarray(inputs["phrase_embeddings"], np.float32))
    ie = _l2n(np.asarray(inputs["input_embeddings"], np.float32))
    bi = np.asarray(inputs["batch_idxs"])
    mi = np.asarray(inputs["phrase_emb_idxs"])
    ki = np.asarray(inputs["input_emb_idxs"])
    rn = np.asarray(inputs["rand_neg_idx"])
    T = float(np.asarray(inputs["temperature"]))
    if cap is None:
        maxc = int(np.bincount(bi, minlength=N).max())
        cap = max(64, ((maxc + 63) // 64) * 64)
    maps, stats = [], []
    for c in range(NCORES):
        m, st = _prep_core(c, cap, pe, ie, bi, mi, ki, rn)
        maps.append(m)
        stats.append(st)
    return maps, stats, cap, T


def kernel(**inputs):
    in_maps, stats, cap, T = make_in_maps(inputs)
    key = (cap, T)
    if key not in _CACHE:
        _CACHE[key] = build_graph(cap, T)
    nc = _CACHE[key]
    res = run_bass_kernel_spmd(nc, in_maps, core_ids=list(range(NCORES)))
    NT = NB * cap // 128
    trip_sum = 0.0
    ce_sum = 0.0
    for c, r in enumerate(res.results):
        of = np.asarray(r["out"], np.float32)            # [128, 9NT]
        t8 = of[:, :8 * NT].reshape(128, NT, 8)
        sumexp = of[:, 8 * NT:9 * NT]                    # [128, NT]
        spos, srnd, vt = stats[c]
        u = np.maximum(t8[:, :, :4] + 1.0 - spos[:, :, None], 0.0)
        s4 = u.sum(-1)
        w = np.maximum(u[:, :, 3], 1.0)
        r2 = np.maximum(srnd + 1.0 - spos[:, :, None], 0.0).sum(-1)
        trip_sum += float(((s4 - w + r2) * vt).sum())
        ce_sum += float(((np.log(sumexp) - T * spos) * vt).sum())
    trip = trip_sum / (P * 5)
    ce = ce_sum / P
    return np.float32(trip), np.float32(ce)
